# revision 2
# baseline (speedup 1.0000x reference)
# kernel.py — Trainium2 Bass kernel for nn_Net_17188459119113 (quantized CNN).
#
# Pipeline (per reference.py):
#   xq = quant4(x); wq = quant4(conv_w)
#   y  = conv2d(xq, wq, VALID) + b; relu; maxpool 4x4/4; flatten
#   fq = quant4(flat); out = fq @ quant4(fc_w).T + fc_b
#
# Data-parallel over 8 NeuronCores (1024 images/core). All heavy math in the
# integer domain (4-bit quantized values are small exact ints), scales applied
# as affine constants at the edges.
#
# Design (vs. the DMA-transpose baseline, 525us -> 236us):
#  - Host supplies x in h-major layout [112=(bq4,h28), 7168=(b256,w28)] f32,
#    so the banded-conv moving tensor is built with 3 contiguous SBUF->SBUF
#    byte-shifted DMA copies per 128-image block (no DRAM round trip, no
#    strided 52B descriptor storms).
#  - Conv = plain fp8 matmuls: k=(dj,h)=84, m=128=(oc16,t2,u4) (i=8c+4t+u),
#    n=384=(b16,j24), 8 matmuls x 3 c-chunks per block. (fp8 DoubleRow and
#    PE 2.4GHz p-state measured as no-ops on this hw; see memory notes.)
#  - j-pool (max over jw=j%4) = DVE tensor_reduce straight out of PSUM.
#  - i-pool (max over u, a partition dim) via DVE StreamTranspose (32x32
#    blocks, SBUF->SBUF) + DVE tensor_tensor max on the now-free u dim.
#    No DMA/PE transposes anywhere.
#  - Second StreamTranspose puts flat features back on partitions in a
#    jj-padded layout; quantization runs with per-partition conv-bias on ACT;
#    FC consumes the result directly as 6 accumulating fp8 matmuls per
#    block-PAIR (n=256) against host-permuted weight slabs (no FC transpose).
#  - Global flat-max via gpsimd partition reduce + AllReduce(max) (64-el
#    payload).
# Output returned as [10, 1024] per core, transposed/concatenated on host.

import numpy as np

P = 128
B_CORE = 1024  # images per core
NB = 8  # b-blocks of 128 images
NCORES = 8
MAGIC = float(np.float32(12582912.0))  # 1.5 * 2**23: fp32 RNE rounding trick

_NC = None  # cached compiled Bass module (input-independent)


def _f32(v):
    return np.float32(v)


def _host_quant_scale(t):
    # mirrors reference _quant scale computation in fp32 arithmetic
    n = _f32(7.0)
    m = np.max(np.abs(t.astype(np.float32))).astype(np.float32)
    return _f32(_f32(m / n) + _f32(1e-8))


# ---- layout helpers (shared by host-const builder and kernel) ----
# conv PSUM partition order: p = oc*8 + t*4 + u  (i = 8c + 4t + u)
# flatT free layout per block: (c3, jh2, imgq4, [jl4, ocl4, t2]=32) = 768
# fqT (post 2nd stream transpose): partition = ocg*32 + (jl*8 + ocl*2 + t),
#   free = (c3, jh2, imgq4, a32); image = imgq*32 + a; oc = ocg*4 + ocl;
#   jj = jh*4 + jl (jl>=2 & jh=1 are pad slots); ii = 2c + t.


def _build_nc():
    import concourse.bass as bass
    import concourse.mybir as mybir
    from concourse import bacc, bass_isa
    from concourse.tile import TileContext

    f32 = mybir.dt.float32
    f16 = mybir.dt.float16
    f8 = mybir.dt.float8e4
    AF = mybir.ActivationFunctionType
    OP = mybir.AluOpType
    DR = mybir.MatmulPerfMode.DoubleRow

    nc = bacc.Bacc(None, num_devices=NCORES)

    # x in h-major layout: [112=(bq4,h28), 7168=(b256,w28)] f32
    x_in = nc.declare_dram_parameter("x", [112, 7168], f32, isOutput=False)
    # banded conv weights: [84=(dj,h), (c3, m128)] fp8
    w3_in = nc.declare_dram_parameter("w3", [84, 384], f8, isOutput=False)
    # permuted FC slabs: [128, 60] fp8 (logical [128,(q3,two2,cls10)])
    fw_in = nc.declare_dram_parameter("fw", [P, 60], f8, isOutput=False)
    # per-partition conv bias in int units (conv_b[oc]/s_xw), fqT partition map
    cbp_in = nc.declare_dram_parameter("cbp", [P, 1], f32, isOutput=False)
    fb_in = nc.declare_dram_parameter("fb", [P, 1], f32, isOutput=False)
    scal_in = nc.declare_dram_parameter("scal", [P, 4], f32, isOutput=False)
    out_ext = nc.declare_dram_parameter("out", [10, B_CORE], f32, isOutput=True)

    cc_in = nc.dram_tensor("cc_in", [1, 64], f32)
    cc_out = nc.dram_tensor("cc_out", [1, 64], f32, addr_space="Shared")

    with TileContext(nc, num_cores=NCORES) as tc:
        with tc.tile_pool(name="const", bufs=1) as cpool:
            w3sb = cpool.tile([84, 384], f8)
            fwsb = cpool.tile([P, 60], f8)
            cbp = cpool.tile([P, 1], f32)
            fb = cpool.tile([P, 1], f32)
            scal = cpool.tile([P, 4], f32)
            lmax = cpool.tile([P, 1], f32)
            magic = cpool.tile([P, 1], f32)
            scal_a = cpool.tile([P, 4], f32)  # DVE-copied (sem-wait hygiene)
            nc.vector.memset(magic[:, :], MAGIC)
            nc.sync.dma_start(out=w3sb[:, :], in_=w3_in[:, :])
            nc.sync.dma_start(out=fwsb[:, :], in_=fw_in[:, :])
            nc.sync.dma_start(out=cbp[:, :], in_=cbp_in[:, :])
            nc.sync.dma_start(out=fb[:, :], in_=fb_in[:, :])
            nc.sync.dma_start(out=scal[:, :], in_=scal_in[:, :])
            nc.vector.memset(lmax[:, :], -3.0e38)
            nc.vector.tensor_copy(out=scal_a[:, :], in_=scal[:, :])

            # xq8 padded by 8 cols so the dj-shifted x3 copies can overrun.
            xq8 = cpool.tile([112, 7176], f8)
            nc.vector.memset(xq8[:, 7168:7176], 0.0)

            rel = []  # per-block-PAIR relu'd flat activations (int units), f32
            with tc.tile_pool(name="rel", bufs=NB // 2) as relpool:
              # ---------- Phase 1: quantize x to fp8 integers ----------
              with tc.tile_pool(name="xf", bufs=2) as xfpool:
                nchunk = 4
                w = 7168 // nchunk  # 1792 els (64 images)
                for q in range(nchunk):
                    sl = slice(q * w, (q + 1) * w)
                    xf = xfpool.tile([112, w], f32)
                    t1 = xfpool.tile([112, w], f32)
                    nc.sync.dma_start(out=xf[:, :], in_=x_in[:, sl])
                    # t1 = x*(1/s_x) + MAGIC (ACT fma; RNE to int grid)
                    nc.scalar.activation(
                        out=t1[:, :], in_=xf[:, :], func=AF.Identity,
                        bias=magic[0:112, 0:1], scale=scal_a[0:112, 0:1],
                    )
                    # xq8 = t1 - MAGIC (exact small ints, cast to fp8).
                    # NOTE: must be DVE — gpsimd runs this ~17x slower.
                    nc.vector.tensor_scalar(
                        out=xq8[:, sl], in0=t1[:, :],
                        scalar1=MAGIC, scalar2=None, op0=OP.subtract,
                    )

              # ---------- Phase 2+3: conv, 2D max-pool, per-block flat ----
              with (
                  tc.tile_pool(name="x3", bufs=2) as x3pool,
                  tc.tile_pool(name="ps", bufs=2, space="PSUM") as pspool,
                  tc.tile_pool(name="yj", bufs=2) as yjpool,
                  tc.tile_pool(name="yt", bufs=2) as ytpool,
                  tc.tile_pool(name="ft", bufs=2) as ftpool,
                  tc.tile_pool(name="sm", bufs=2) as smpool,
              ):
                for bb in range(NB):
                    q28 = (bb // 2) * 28
                    base = (bb % 2) * 3584
                    # x3 [84=(dj,h), (b128, w28)] fp8: dj-shifted copies
                    x3 = x3pool.tile([84, 3584], f8)
                    src = xq8[q28:q28 + 28, :]
                    for dj in range(3):
                        nc.sync.dma_start(
                            out=x3[28 * dj:28 * (dj + 1), :],
                            in_=src[:, base + dj:base + dj + 3584])

                    x3v = x3[:, :].rearrange("p (b w) -> p b w", w=28)
                    ft = ftpool.tile([P, 768], f16)
                    # zero pad slots (c, jh=1, imgq, jl in {2,3}, oclt)
                    nc.vector.memset(
                        ft[:, :].rearrange(
                            "p (c jh imgq jl oclt) -> p c jh imgq jl oclt",
                            c=3, jh=2, imgq=4, jl=4)[:, :, 1, :, 2:4, :],
                        0.0)

                    for c in range(3):
                        yj = yjpool.tile([P, 768], f16)
                        for bsq in range(2):
                            ps = pspool.tile([P, 2048], f32, tag="ps")
                            for g in range(4):
                                bs = bsq * 4 + g
                                # rhs [84, 16, 24]: imgs bs*16.., j 0..23
                                rhs = x3v[:, bs * 16:(bs + 1) * 16, 0:24]
                                nc.tensor.matmul(
                                    out=ps[:, g * 512:g * 512 + 384],
                                    lhsT=w3sb[:, c * 128:(c + 1) * 128],
                                    rhs=rhs, start=True, stop=True,
                                )
                            # j-pool: max over jw=j%4, PSUM -> SBUF fp16
                            pin = ps[:, :].rearrange(
                                "p (g s) -> p g s", g=4)[:, :, 0:384].rearrange(
                                "p g (b jj jw) -> p g b jj jw", b=16, jj=6, jw=4)
                            yout = yj[:, :].rearrange(
                                "p (jj half g b) -> p half g b jj",
                                jj=6, half=2, g=4, b=16)[:, bsq]
                            nc.vector.tensor_reduce(
                                out=yout, in_=pin,
                                axis=mybir.AxisListType.X, op=OP.max,
                            )
                        # i-pool part 1: StreamTranspose 32x32 blocks
                        # yj [p=(oc16,t2,u4), (jj6, img128)] ->
                        # yt [p=(ocg4,a32), (jj6, imgq4, (ocl4,t2,u4)=32)]
                        yt = ytpool.tile([P, 768], f16)
                        nc.vector.transpose(out=yt[:, :], in_=yj[:, :])
                        # i-pool part 2: max over u (now free) on Pool engine
                        tv = yt[:, :].rearrange(
                            "p (jj imgq oclt u) -> p jj imgq oclt u",
                            jj=6, imgq=4, oclt=8)
                        t1 = smpool.tile([P, 384], f16, tag="t1")
                        t1v = t1[:, :].rearrange(
                            "p (jj imgq oclt s) -> p jj imgq oclt s",
                            jj=6, imgq=4, oclt=8)
                        nc.vector.tensor_tensor(
                            t1v[:, :, :, :, :], tv[:, :, :, :, 0:2],
                            tv[:, :, :, :, 2:4], OP.max)
                        # final u-max into flatT slots (jh-split for affine APs)
                        ftv = ft[:, :].rearrange(
                            "p (c jh imgq jl oclt) -> p c jh imgq jl oclt",
                            c=3, jh=2, imgq=4, jl=4)
                        t1a = t1[:, :].rearrange(
                            "p (jj imgq oclt s) -> p jj imgq oclt s",
                            jj=6, imgq=4, oclt=8)
                        # jh=0: jj 0..3 -> jl 0..3
                        nc.vector.tensor_tensor(
                            ftv[:, c, 0, :, :, :].rearrange(
                                "p imgq jl oclt -> p jl imgq oclt"),
                            t1a[:, 0:4, :, :, 0],
                            t1a[:, 0:4, :, :, 1], OP.max)
                        # jh=1: jj 4..5 -> jl 0..1
                        nc.vector.tensor_tensor(
                            ftv[:, c, 1, :, 0:2, :].rearrange(
                                "p imgq jl oclt -> p jl imgq oclt"),
                            t1a[:, 4:6, :, :, 0],
                            t1a[:, 4:6, :, :, 1], OP.max)

                    # 2nd StreamTranspose: features onto partitions
                    # ft [p=(ocg4,a32), (c,jh,imgq, phi32)] ->
                    # fqT [p=(ocg4,phi32), (c,jh,imgq, a32)]
                    fqt = ftpool.tile([P, 768], f16, tag="fqt")
                    nc.vector.transpose(out=fqt[:, :], in_=ft[:, :])
                    # relu(v + bias) in int units, f32 (exact; persists).
                    # blocks 2k/2k+1 share one [P, 1536] tile (halves) so the
                    # FC can run 256-image matmuls per pair.
                    if bb % 2 == 0:
                        rpair = relpool.tile([P, 1536], f32)
                        rel.append(rpair)
                    rb = rel[-1][:, (bb % 2) * 768:(bb % 2) * 768 + 768]
                    nc.scalar.activation(
                        out=rb, in_=fqt[:, :], func=AF.Relu,
                        bias=cbp[:, 0:1], scale=1.0,
                    )
                    # running local max (relu'd, so >= 0)
                    tmp = smpool.tile([P, 1], f32, tag="lm")
                    nc.vector.tensor_reduce(
                        out=tmp[:, :], in_=rb,
                        axis=mybir.AxisListType.X, op=OP.max,
                    )
                    nc.vector.tensor_tensor(
                        lmax[:, :], lmax[:, :], tmp[:, :], OP.max)

              # ---------- Phase 4: global scale via AllReduce(max) ----------
              with tc.tile_pool(name="ar", bufs=1) as arpool:
                lmr = arpool.tile([P, 1], f32)
                nc.gpsimd.partition_all_reduce(
                    lmr[:, :], lmax[:, :], 128, bass_isa.ReduceOp.max)
                bc = arpool.tile([1, 64], f32)
                nc.vector.tensor_copy(
                    out=bc[:, :], in_=lmr[0:1, 0:1].to_broadcast((1, 64)))
                nc.gpsimd.dma_start(out=cc_in[:, :], in_=bc[:, :])
                nc.gpsimd.collective_compute(
                    "AllReduce", OP.max,
                    replica_groups=[list(range(NCORES))],
                    ins=[cc_in[:, :]], outs=[cc_out[:, :]],
                )
                gm = arpool.tile([1, 1], f32)
                nc.gpsimd.dma_start(out=gm[:, :], in_=cc_out[0:1, 0:1])
                gmb = arpool.tile([P, 1], f32)
                nc.gpsimd.partition_broadcast(gmb[:, :], gm[:, :], channels=P)
                # gmax >= 0 already (relu'd, int units); to real units first.
                gmr = arpool.tile([P, 1], f32)
                nc.vector.tensor_scalar(
                    out=gmr[:, :], in0=gmb[:, :],
                    scalar1=scal_a[:, 1:2], scalar2=None, op0=OP.mult,
                )
                # s_f = gmax_real/7 + 1e-8  (matches reference fp32 math)
                sf = arpool.tile([P, 1], f32)
                nc.vector.tensor_scalar(
                    out=sf[:, :], in0=gmr[:, :],
                    scalar1=float(np.float32(1.0) / np.float32(7.0)),
                    scalar2=float(np.float32(1e-8)),
                    op0=OP.mult, op1=OP.add,
                )
                invsf = arpool.tile([P, 1], f32)
                nc.vector.reciprocal(out=invsf[:, :], in_=sf[:, :])
                # alpha = s_xw / s_f  (rel is in int units)
                alpha = arpool.tile([P, 1], f32)
                nc.vector.tensor_scalar(
                    out=alpha[:, :], in0=invsf[:, :],
                    scalar1=scal_a[:, 1:2], scalar2=None, op0=OP.mult,
                )
                # sprod = s_f * s_fw (output scale)
                sprod = arpool.tile([P, 1], f32)
                nc.vector.tensor_scalar(
                    out=sprod[:, :], in0=sf[:, :],
                    scalar1=scal_a[:, 2:3], scalar2=None, op0=OP.mult,
                )

                # ---------- Phase 5: quantize + FC ----------
                with (
                    tc.tile_pool(name="fq", bufs=2) as fqpool,
                    tc.tile_pool(name="psfc", bufs=2, space="PSUM") as pfcpool,
                    tc.tile_pool(name="outp", bufs=2) as outpool,
                ):
                    for pr in range(NB // 2):
                        rb = rel[pr]
                        # r2 = rel*alpha + MAGIC  (ACT fma -> RNE round)
                        r2 = fqpool.tile([P, 1536], f32, tag="r2")
                        nc.scalar.activation(
                            out=r2[:, :], in_=rb[:, :], func=AF.Identity,
                            bias=magic[:, 0:1], scale=alpha[:, 0:1],
                        )
                        fq8 = fqpool.tile([P, 1536], f8, tag="fq8")
                        nc.vector.tensor_scalar(
                            out=fq8[:, :], in0=r2[:, :],
                            scalar1=MAGIC, scalar2=None, op0=OP.subtract,
                        )
                        psfc = pfcpool.tile([10, 256], f32)
                        # rhs per chunk: [128, (blk2, n128)] (blk-stride 768)
                        fqv = fq8[:, :].rearrange(
                            "p (blk ch n) -> p ch blk n", blk=2, ch=6)
                        fwv = fwsb[:, :].rearrange(
                            "p (ch cls) -> p ch cls", ch=6)
                        for ch in range(6):
                            nc.tensor.matmul(
                                out=psfc[:, :],
                                lhsT=fwv[:, ch],
                                rhs=fqv[:, ch],
                                start=(ch == 0), stop=(ch == 5),
                            )
                        osb = outpool.tile([10, 256], f32)
                        nc.scalar.activation(
                            out=osb[:, :], in_=psfc[:, :], func=AF.Identity,
                            bias=fb[0:10, 0:1], scale=sprod[0:10, 0:1],
                        )
                        nc.sync.dma_start(
                            out=out_ext[:, pr * 256:(pr + 1) * 256],
                            in_=osb[:, :])

    nc.finalize()
    return nc


def _host_constants(x, conv_w, conv_b, fc_w, fc_b):
    s_x = _host_quant_scale(x)
    s_w = _host_quant_scale(conv_w)
    s_fw = _host_quant_scale(fc_w)
    kw = np.round(conv_w.astype(np.float32) / s_w).astype(np.float32)
    kfw = np.round(fc_w.astype(np.float32) / s_fw).astype(np.float32)

    # Banded conv weights [84=(dj,h), (c, m)].
    # m column (per c): oc*8 + t*4 + u; i = 8c + 4t + u; di = h - i in [0,3).
    w3 = np.zeros((84, 3, 128), np.float32)
    for dj in range(3):
        for h in range(28):
            r = 28 * dj + h
            for c in range(3):
                for t in range(2):
                    for u in range(4):
                        i = 8 * c + 4 * t + u
                        di = h - i
                        if 0 <= di <= 2:
                            for oc in range(16):
                                w3[r, c, oc * 8 + t * 4 + u] = \
                                    kw[oc, 0, di, dj]
    w3 = w3.reshape(84, 384)

    # FC slabs permuted to the fqT layout.
    # fqT partition p = ocg*32 + jl*8 + ocl*2 + t ; chunk ch = c*2 + jh.
    # feature flat idx (torch flatten) = oc*36 + ii*6 + jj,
    #   oc = ocg*4 + ocl, ii = 2c + t, jj = jh*4 + jl (pad if jj >= 6).
    fw = np.zeros((128, 3, 2, 10), np.float32)
    for p in range(128):
        ocg, rem = divmod(p, 32)
        jl, rem2 = divmod(rem, 8)
        ocl, t = divmod(rem2, 2)
        oc = ocg * 4 + ocl
        for ch in range(6):
            c, jh = divmod(ch, 2)
            jj = jh * 4 + jl
            if jj < 6:
                k = oc * 36 + (2 * c + t) * 6 + jj
                fw[p, c, jh, :] = kfw[:, k]
    fw = fw.reshape(128, 60)

    # per-partition conv bias in int units (pad rows 0)
    cbp = np.zeros((128, 1), np.float32)
    s_xw = _f32(s_x * s_w)
    for p in range(128):
        ocg, rem = divmod(p, 32)
        jl, rem2 = divmod(rem, 8)
        ocl, t = divmod(rem2, 2)
        oc = ocg * 4 + ocl
        cbp[p, 0] = _f32(conv_b[oc] / s_xw)

    fb = np.zeros((P, 1), np.float32)
    fb[:10, 0] = fc_b.astype(np.float32)

    inv_sx = _f32(_f32(1.0) / s_x)
    scal = np.tile(
        np.array([inv_sx, s_xw, s_fw, 0.0], np.float32)[None, :], (P, 1))

    import ml_dtypes
    return {
        "w3": w3.astype(ml_dtypes.float8_e4m3),
        "fw": fw.astype(ml_dtypes.float8_e4m3),
        "cbp": cbp,
        "fb": fb,
        "scal": scal.astype(np.float32),
    }


def _get_nc():
    global _NC
    if _NC is None:
        _NC = _build_nc()
    return _NC


def kernel(x, conv_w, conv_b, fc_w, fc_b, _trace=False):
    from concourse.bass_utils import run_bass_kernel_spmd

    x = np.asarray(x, np.float32)
    consts = _host_constants(
        x, np.asarray(conv_w, np.float32), np.asarray(conv_b, np.float32),
        np.asarray(fc_w, np.float32), np.asarray(fc_b, np.float32))

    nc = _get_nc()
    in_maps = []
    for cix in range(NCORES):
        shard = x[cix * B_CORE:(cix + 1) * B_CORE]  # [1024,1,28,28]
        # h-major: [bq4, h28, b256, w28] -> [112, 7168]
        xh = shard.reshape(4, 256, 28, 28).transpose(0, 2, 1, 3)
        m = {"x": np.ascontiguousarray(xh.reshape(112, 7168))}
        m.update(consts)
        in_maps.append(m)

    res = run_bass_kernel_spmd(
        nc, in_maps, list(range(NCORES)), trace=_trace,
        trace_cores=list(range(NCORES)) if _trace else None)
    out = np.concatenate([r["out"].T for r in res.results], axis=0)
    if _trace:
        kernel._last_results = res
    return np.ascontiguousarray(out.astype(np.float32))

